# revision 21
# baseline (speedup 1.0000x reference)
"""Trainium2 Bass kernel for batched 1-NN centroid distance (cdist + top-1 + softmin).

Math: score[b, n] = min_p sqrt(||e_bn||^2 + ||c_p||^2 - 2 e_bn . c_p)
with embeds [8, 3136, 1536], centroids [4096, 1536].

Sharding: data-parallel over B; core b handles embeds[b] vs the full
replicated centroid bank.

Per-core device pipeline (centroid-block-outer to keep the PE dense from
~6us in while B streams):
  PE     : acc[n, p] = sum_k (-2 E)^T_k . C_k   (bf16 or fp8-DoubleRow, fp32 PSUM)
  DVE    : acc += c2 (in place on PSUM) ; blockmin = min_free(acc)
  DVE    : min over the 8 block mins
  ACT    : out = sqrt(min + e2)

Norm terms ||e||^2 / ||c||^2 are computed exactly in fp32 on the host, so
the only low-precision term is the cross product. Measured vs the fp32
reference: fp8 (default) max rel err 2.5e-3 @ 285us HW; bf16 1.3e-4 @
542us HW — flip DEFAULT_DTYPE to "bf16" if tighter accuracy is needed.
"""

import numpy as np
import ml_dtypes
from contextlib import ExitStack

import concourse.bass as bass
import concourse.mybir as mybir
import concourse.tile as tile
from concourse import bacc
from concourse.bass import ts
from concourse.bass_utils import run_bass_kernel_spmd

F32 = mybir.dt.float32

# Problem shape (hardcoded per contract)
B, N, D, P = 8, 3136, 1536, 4096
H = 56
NCORES = 8

# Tiling
PART = 128
NPAD = 3200                 # 25 * 128 token padding
TT = NPAD // PART           # 25 token tiles
KT = D // PART              # 12 contraction chunks
PBLK = 512                  # centroid block (one PSUM bank of fp32)
PB = P // PBLK              # 8 centroid blocks
PB_STEP = 2                 # centroid blocks per outer step (B quarter)

DEFAULT_DTYPE = "fp8"

_NP_DT = {
    "bf16": np.dtype(ml_dtypes.bfloat16),
    "fp8": np.dtype(ml_dtypes.float8_e4m3),
}
_MY_DT = {
    "bf16": mybir.dt.bfloat16,
    "fp8": mybir.dt.float8e4,
}


def build_nc(tt=TT, kt=KT, pb=PB, pblk=PBLK, dtype=DEFAULT_DTYPE):
    """Build the single-core Bass program (same program runs SPMD on all cores)."""
    DT = _MY_DT[dtype]
    fp8 = dtype == "fp8"
    kstep = 2 if fp8 else 1
    perf_mode = mybir.MatmulPerfMode.DoubleRow if fp8 else None
    steps = max(1, pb // PB_STEP)
    pbs = pb // steps                    # centroid blocks per step
    bw = pbs * pblk                      # B step width in centroids

    nc = bacc.Bacc(None, target_bir_lowering=False)
    A = nc.dram_tensor("a", [tt, PART, kt, PART], DT, kind="ExternalInput")
    B2 = nc.dram_tensor("b", [steps, kt, PART, bw], DT, kind="ExternalInput")
    C2 = nc.dram_tensor("c2", [PART, pb * pblk], F32, kind="ExternalInput")
    E2 = nc.dram_tensor("e2", [PART, tt], F32, kind="ExternalInput")
    OUT = nc.dram_tensor("out", [PART, tt], F32, kind="ExternalOutput")

    with ExitStack() as ctx:
        tc = ctx.enter_context(tile.TileContext(nc))
        const = ctx.enter_context(tc.tile_pool(name="const", bufs=1))
        c2_sb = const.tile([PART, pb * pblk], F32)
        e2_sb = const.tile([PART, tt], F32)
        out_sb = const.tile([PART, tt], F32)

        a_pool = ctx.enter_context(tc.tile_pool(name="apool", bufs=tt))
        b_pool = ctx.enter_context(tc.tile_pool(name="bpool", bufs=2))
        psum_pool = ctx.enter_context(tc.tile_pool(name="psum", bufs=4, space="PSUM"))
        mb_pool = ctx.enter_context(tc.tile_pool(name="mb", bufs=tt))
        m1_pool = ctx.enter_context(tc.tile_pool(name="m1", bufs=2))
        scr_pool = ctx.enter_context(tc.tile_pool(name="scr", bufs=3))

        a_tiles = [None] * tt
        minblks = [None] * tt

        # First token tiles ahead of the B stream so the PE can start ASAP.
        for t in range(min(2, tt)):
            a_tiles[t] = a_pool.tile([PART, kt, PART], DT, name="a_t", tag="a_t")
            nc.sync.dma_start(a_tiles[t][:], A[t])

        for q in range(steps):
            bq = b_pool.tile([PART, kt, bw], DT)
            for k in range(kt):
                nc.sync.dma_start(bq[:, k, :], B2[q, k])
            # c2 slice for this step only — keeps the big c2 DMA from
            # delaying the a-tile stream at startup.
            nc.sync.dma_start(c2_sb[:, q * bw : (q + 1) * bw],
                              C2[:, q * bw : (q + 1) * bw])
            if q == min(1, steps - 1):
                nc.sync.dma_start(e2_sb[:], E2[:])
            for t in range(tt):
                if q == 0:
                    if a_tiles[t] is None:
                        a_tiles[t] = a_pool.tile([PART, kt, PART], DT, name="a_t", tag="a_t")
                        nc.sync.dma_start(a_tiles[t][:], A[t])
                    minblks[t] = mb_pool.tile([PART, steps], F32, name="mb", tag="mb")
                a_t = a_tiles[t]
                # One 2-bank PSUM tile per (t, step): both centroid blocks of
                # the step land side by side, drained by ONE add + ONE min.
                ps = psum_pool.tile([PART, bw], F32)
                for j in range(pbs):
                    for kk in range(0, kt, kstep):
                        if fp8:
                            lhs = a_t[:, kk : kk + 2, :]
                            rhs = bq[:, kk : kk + 2, ts(j, pblk)]
                        else:
                            lhs = a_t[:, kk, :]
                            rhs = bq[:, kk, ts(j, pblk)]
                        nc.tensor.matmul(
                            ps[:, ts(j, pblk)], lhs, rhs,
                            start=(kk == 0),
                            stop=(kk + kstep >= kt),
                            perf_mode=perf_mode,
                        )
                # acc + c2_step -> SBUF scratch (frees the PSUM banks right
                # after the add instead of after the reduce, killing the
                # periodic PE stall on PSUM recycle); stepmin = min_free
                sc = scr_pool.tile([PART, bw], F32, name="sc", tag="sc")
                nc.vector.tensor_add(sc[:], ps[:], c2_sb[:, q * bw : (q + 1) * bw])
                nc.vector.tensor_reduce(
                    minblks[t][:, q : q + 1],
                    sc[:],
                    axis=mybir.AxisListType.X,
                    op=mybir.AluOpType.min,
                )
                if q == steps - 1:
                    m1 = m1_pool.tile([PART, 1], F32)
                    nc.vector.tensor_reduce(
                        m1[:], minblks[t][:], axis=mybir.AxisListType.X,
                        op=mybir.AluOpType.min,
                    )
                    nc.scalar.activation(
                        out_sb[:, t : t + 1], m1[:],
                        mybir.ActivationFunctionType.Sqrt,
                        bias=e2_sb[:, t : t + 1], scale=1.0,
                    )
        nc.sync.dma_start(OUT[:], out_sb[:])
    nc.compile()
    return nc


def prep_inputs(embeds, centroids, dtype=DEFAULT_DTYPE):
    """Host-side prep: scale/cast/layout so the device only does matmul+min+sqrt."""
    npdt = _NP_DT[dtype]
    steps = PB // PB_STEP
    bw = PB_STEP * PBLK

    e = np.ascontiguousarray(embeds, dtype=np.float32)
    c = np.ascontiguousarray(centroids, dtype=np.float32)

    # A[core][t, dp, k, m] = -2 * e[core, t*128+m, k*128+dp]
    ep = np.zeros((B, NPAD, D), dtype=npdt)
    ep[:, :N] = (e * -2.0).astype(npdt)
    a_all = np.ascontiguousarray(
        ep.reshape(B, TT, PART, KT, PART).transpose(0, 1, 4, 3, 2)
    )

    # B2[q, k, dp, j*512+i] = c[(q*PB_STEP+j)*512 + i, k*128 + dp]
    b2 = np.ascontiguousarray(
        c.reshape(steps, PB_STEP, PBLK, KT, PART)
        .transpose(0, 3, 4, 1, 2)
        .reshape(steps, KT, PART, bw)
        .astype(npdt)
    )

    # c2 broadcast to all partitions (fp32)
    c2 = np.einsum("pd,pd->p", c, c, dtype=np.float32)
    c2b = np.ascontiguousarray(np.broadcast_to(c2, (PART, P)))

    # e2[core][p, t] = ||e[core, t*128+p]||^2, padded tokens -> 0
    e2 = np.zeros((B, NPAD), dtype=np.float32)
    e2[:, :N] = np.einsum("bnd,bnd->bn", e, e, dtype=np.float32)
    e2t = np.ascontiguousarray(e2.reshape(B, TT, PART).transpose(0, 2, 1))

    return [
        {"a": a_all[b], "b": b2, "c2": c2b, "e2": e2t[b]} for b in range(B)
    ]


_NC_CACHE = {}


def _get_nc(dtype):
    if dtype not in _NC_CACHE:
        _NC_CACHE[dtype] = build_nc(dtype=dtype)
    return _NC_CACHE[dtype]


def run(inputs, trace=False, dtype=DEFAULT_DTYPE, **kwargs):
    """Returns ((loss, score), BassKernelResults)."""
    nc = _get_nc(dtype)
    in_maps = prep_inputs(inputs["embeds"], inputs["centroids"], dtype=dtype)
    res = run_bass_kernel_spmd(nc, in_maps, core_ids=list(range(NCORES)),
                               trace=trace, **kwargs)
    outs = np.stack([np.asarray(res.results[i]["out"]) for i in range(NCORES)])
    # out[p, t] -> token index t*128 + p
    toks = outs.transpose(0, 2, 1).reshape(B, NPAD)[:, :N]
    score = toks.reshape(B, 1, H, H).astype(np.float32)
    loss = np.float32(0.0)
    return (loss, score), res


def kernel(**inputs):
    (loss, score), _ = run(inputs)
    return loss, score


# revision 25
# speedup vs baseline: 1.0008x; 1.0008x over previous
"""Trainium2 Bass kernel for batched 1-NN centroid distance (cdist + top-1 + softmin).

Math: score[b, n] = min_p sqrt(||e_bn||^2 + ||c_p||^2 - 2 e_bn . c_p)
with embeds [8, 3136, 1536], centroids [4096, 1536].

Sharding: data-parallel over B; core b handles embeds[b] vs the full
replicated centroid bank.

Per-core device pipeline (centroid-block-outer to keep the PE dense from
~6us in while B streams):
  PE     : acc[n, p] = sum_k (-2 E)^T_k . C_k   (bf16 or fp8-DoubleRow, fp32 PSUM)
  DVE    : acc += c2 (in place on PSUM) ; blockmin = min_free(acc)
  DVE    : min over the 8 block mins
  ACT    : out = sqrt(min + e2)

Norm terms ||e||^2 / ||c||^2 are computed exactly in fp32 on the host, so
the only low-precision term is the cross product. Measured vs the fp32
reference: fp8 (default) max rel err 2.5e-3 @ 285us HW; bf16 1.3e-4 @
542us HW — flip DEFAULT_DTYPE to "bf16" if tighter accuracy is needed.
"""

import numpy as np
import ml_dtypes
from contextlib import ExitStack

import concourse.bass as bass
import concourse.mybir as mybir
import concourse.tile as tile
from concourse import bacc
from concourse.bass import ts
from concourse.bass_utils import run_bass_kernel_spmd

F32 = mybir.dt.float32

# Problem shape (hardcoded per contract)
B, N, D, P = 8, 3136, 1536, 4096
H = 56
NCORES = 8

# Tiling
PART = 128
NPAD = 3200                 # 25 * 128 token padding
TT = NPAD // PART           # 25 token tiles
KT = D // PART              # 12 contraction chunks
PBLK = 512                  # centroid block (one PSUM bank of fp32)
PB = P // PBLK              # 8 centroid blocks
PB_STEP = 2                 # centroid blocks per outer step (B quarter)

DEFAULT_DTYPE = "fp8"

_NP_DT = {
    "bf16": np.dtype(ml_dtypes.bfloat16),
    "fp8": np.dtype(ml_dtypes.float8_e4m3),
}
_MY_DT = {
    "bf16": mybir.dt.bfloat16,
    "fp8": mybir.dt.float8e4,
}


def build_nc(tt=TT, kt=KT, pb=PB, pblk=PBLK, dtype=DEFAULT_DTYPE):
    """Build the single-core Bass program (same program runs SPMD on all cores)."""
    DT = _MY_DT[dtype]
    fp8 = dtype == "fp8"
    kstep = 2 if fp8 else 1
    perf_mode = mybir.MatmulPerfMode.DoubleRow if fp8 else None
    steps = max(1, pb // PB_STEP)
    pbs = pb // steps                    # centroid blocks per step
    bw = pbs * pblk                      # B step width in centroids

    nc = bacc.Bacc(None, target_bir_lowering=False)
    A = nc.dram_tensor("a", [tt, PART, kt, PART], DT, kind="ExternalInput")
    B2 = nc.dram_tensor("b", [steps, kt, PART, bw], DT, kind="ExternalInput")
    C2 = nc.dram_tensor("c2", [PART, pb * pblk], F32, kind="ExternalInput")
    E2 = nc.dram_tensor("e2", [PART, tt], F32, kind="ExternalInput")
    OUT = nc.dram_tensor("out", [PART, tt], F32, kind="ExternalOutput")

    with ExitStack() as ctx:
        tc = ctx.enter_context(tile.TileContext(nc))
        const = ctx.enter_context(tc.tile_pool(name="const", bufs=1))
        c2_sb = const.tile([PART, pb * pblk], F32)
        e2_sb = const.tile([PART, tt], F32)
        out_sb = const.tile([PART, tt], F32)

        a_pool = ctx.enter_context(tc.tile_pool(name="apool", bufs=tt))
        b_pool = ctx.enter_context(tc.tile_pool(name="bpool", bufs=2))
        psum_pool = ctx.enter_context(tc.tile_pool(name="psum", bufs=4, space="PSUM"))
        mb_pool = ctx.enter_context(tc.tile_pool(name="mb", bufs=tt))
        m1_pool = ctx.enter_context(tc.tile_pool(name="m1", bufs=2))
        scr_pool = ctx.enter_context(tc.tile_pool(name="scr", bufs=3))

        a_tiles = [None] * tt
        minblks = [None] * tt

        # First token tiles ahead of the B stream so the PE can start ASAP.
        for t in range(min(2, tt)):
            a_tiles[t] = a_pool.tile([PART, kt, PART], DT, name="a_t", tag="a_t")
            nc.sync.dma_start(a_tiles[t][:], A[t])

        for q in range(steps):
            bq = b_pool.tile([PART, kt, bw], DT)
            # One batched strided DMA for the whole B step (fewer descriptor
            # round-trips than 12 row DMAs in the serialized startup window).
            nc.sync.dma_start(bq[:], B2[q].rearrange("k p w -> p k w"))
            nc.sync.dma_start(c2_sb[:, q * bw : (q + 1) * bw],
                              C2[:, q * bw : (q + 1) * bw])
            if q == min(1, steps - 1):
                nc.sync.dma_start(e2_sb[:], E2[:])
            for t in range(tt):
                if q == 0:
                    if a_tiles[t] is None:
                        a_tiles[t] = a_pool.tile([PART, kt, PART], DT, name="a_t", tag="a_t")
                        nc.sync.dma_start(a_tiles[t][:], A[t])
                    minblks[t] = mb_pool.tile([PART, steps], F32, name="mb", tag="mb")
                a_t = a_tiles[t]
                # One 2-bank PSUM tile per (t, step): both centroid blocks of
                # the step land side by side, drained by ONE add + ONE min.
                ps = psum_pool.tile([PART, bw], F32)
                for j in range(pbs):
                    for kk in range(0, kt, kstep):
                        if fp8:
                            lhs = a_t[:, kk : kk + 2, :]
                            rhs = bq[:, kk : kk + 2, ts(j, pblk)]
                        else:
                            lhs = a_t[:, kk, :]
                            rhs = bq[:, kk, ts(j, pblk)]
                        nc.tensor.matmul(
                            ps[:, ts(j, pblk)], lhs, rhs,
                            start=(kk == 0),
                            stop=(kk + kstep >= kt),
                            perf_mode=perf_mode,
                        )
                # acc + c2_step -> SBUF scratch (frees the PSUM banks right
                # after the add instead of after the reduce, killing the
                # periodic PE stall on PSUM recycle); stepmin = min_free
                sc = scr_pool.tile([PART, bw], F32, name="sc", tag="sc")
                nc.vector.tensor_add(sc[:], ps[:], c2_sb[:, q * bw : (q + 1) * bw])
                nc.vector.tensor_reduce(
                    minblks[t][:, q : q + 1],
                    sc[:],
                    axis=mybir.AxisListType.X,
                    op=mybir.AluOpType.min,
                )
                if q == steps - 1:
                    m1 = m1_pool.tile([PART, 1], F32)
                    nc.vector.tensor_reduce(
                        m1[:], minblks[t][:], axis=mybir.AxisListType.X,
                        op=mybir.AluOpType.min,
                    )
                    nc.scalar.activation(
                        out_sb[:, t : t + 1], m1[:],
                        mybir.ActivationFunctionType.Sqrt,
                        bias=e2_sb[:, t : t + 1], scale=1.0,
                    )
        nc.sync.dma_start(OUT[:], out_sb[:])
    nc.compile()
    return nc


def prep_inputs(embeds, centroids, dtype=DEFAULT_DTYPE):
    """Host-side prep: scale/cast/layout so the device only does matmul+min+sqrt."""
    npdt = _NP_DT[dtype]
    steps = PB // PB_STEP
    bw = PB_STEP * PBLK

    e = np.ascontiguousarray(embeds, dtype=np.float32)
    c = np.ascontiguousarray(centroids, dtype=np.float32)

    # A[core][t, dp, k, m] = -2 * e[core, t*128+m, k*128+dp]
    ep = np.zeros((B, NPAD, D), dtype=npdt)
    ep[:, :N] = (e * -2.0).astype(npdt)
    a_all = np.ascontiguousarray(
        ep.reshape(B, TT, PART, KT, PART).transpose(0, 1, 4, 3, 2)
    )

    # B2[q, k, dp, j*512+i] = c[(q*PB_STEP+j)*512 + i, k*128 + dp]
    b2 = np.ascontiguousarray(
        c.reshape(steps, PB_STEP, PBLK, KT, PART)
        .transpose(0, 3, 4, 1, 2)
        .reshape(steps, KT, PART, bw)
        .astype(npdt)
    )

    # c2 broadcast to all partitions (fp32)
    c2 = np.einsum("pd,pd->p", c, c, dtype=np.float32)
    c2b = np.ascontiguousarray(np.broadcast_to(c2, (PART, P)))

    # e2[core][p, t] = ||e[core, t*128+p]||^2, padded tokens -> 0
    e2 = np.zeros((B, NPAD), dtype=np.float32)
    e2[:, :N] = np.einsum("bnd,bnd->bn", e, e, dtype=np.float32)
    e2t = np.ascontiguousarray(e2.reshape(B, TT, PART).transpose(0, 2, 1))

    return [
        {"a": a_all[b], "b": b2, "c2": c2b, "e2": e2t[b]} for b in range(B)
    ]


_NC_CACHE = {}


def _get_nc(dtype):
    if dtype not in _NC_CACHE:
        _NC_CACHE[dtype] = build_nc(dtype=dtype)
    return _NC_CACHE[dtype]


def run(inputs, trace=False, dtype=DEFAULT_DTYPE, **kwargs):
    """Returns ((loss, score), BassKernelResults)."""
    nc = _get_nc(dtype)
    in_maps = prep_inputs(inputs["embeds"], inputs["centroids"], dtype=dtype)
    res = run_bass_kernel_spmd(nc, in_maps, core_ids=list(range(NCORES)),
                               trace=trace, **kwargs)
    outs = np.stack([np.asarray(res.results[i]["out"]) for i in range(NCORES)])
    # out[p, t] -> token index t*128 + p
    toks = outs.transpose(0, 2, 1).reshape(B, NPAD)[:, :N]
    score = toks.reshape(B, 1, H, H).astype(np.float32)
    loss = np.float32(0.0)
    return (loss, score), res


def kernel(**inputs):
    (loss, score), _ = run(inputs)
    return loss, score


# revision 26
# speedup vs baseline: 1.0055x; 1.0046x over previous
"""Trainium2 Bass kernel for batched 1-NN centroid distance (cdist + top-1 + softmin).

Math: score[b, n] = min_p sqrt(||e_bn||^2 + ||c_p||^2 - 2 e_bn . c_p)
with embeds [8, 3136, 1536], centroids [4096, 1536].

Sharding: data-parallel over B; core b handles embeds[b] vs the full
replicated centroid bank.

Per-core device pipeline (centroid-block-outer to keep the PE dense from
~6us in while B streams):
  PE     : acc[n, p] = sum_k (-2 E)^T_k . C_k   (bf16 or fp8-DoubleRow, fp32 PSUM)
  DVE    : acc += c2 (in place on PSUM) ; blockmin = min_free(acc)
  DVE    : min over the 8 block mins
  ACT    : out = sqrt(min + e2)

Norm terms ||e||^2 / ||c||^2 are computed exactly in fp32 on the host, so
the only low-precision term is the cross product. Measured vs the fp32
reference: fp8 (default) max rel err 2.5e-3 @ 285us HW; bf16 1.3e-4 @
542us HW — flip DEFAULT_DTYPE to "bf16" if tighter accuracy is needed.
"""

import numpy as np
import ml_dtypes
from contextlib import ExitStack

import concourse.bass as bass
import concourse.mybir as mybir
import concourse.tile as tile
from concourse import bacc
from concourse.bass import ts
from concourse.bass_utils import run_bass_kernel_spmd

F32 = mybir.dt.float32

# Problem shape (hardcoded per contract)
B, N, D, P = 8, 3136, 1536, 4096
H = 56
NCORES = 8

# Tiling
PART = 128
NPAD = 3200                 # 25 * 128 token padding
TT = NPAD // PART           # 25 token tiles
KT = D // PART              # 12 contraction chunks
PBLK = 512                  # centroid block (one PSUM bank of fp32)
PB = P // PBLK              # 8 centroid blocks
PB_STEP = 2                 # centroid blocks per outer step (B quarter)

DEFAULT_DTYPE = "fp8"

_NP_DT = {
    "bf16": np.dtype(ml_dtypes.bfloat16),
    "fp8": np.dtype(ml_dtypes.float8_e4m3),
}
_MY_DT = {
    "bf16": mybir.dt.bfloat16,
    "fp8": mybir.dt.float8e4,
}


def build_nc(tt=TT, kt=KT, pb=PB, pblk=PBLK, dtype=DEFAULT_DTYPE):
    """Build the single-core Bass program (same program runs SPMD on all cores)."""
    DT = _MY_DT[dtype]
    fp8 = dtype == "fp8"
    kstep = 2 if fp8 else 1
    perf_mode = mybir.MatmulPerfMode.DoubleRow if fp8 else None
    steps = max(1, pb // PB_STEP)
    pbs = pb // steps                    # centroid blocks per step
    bw = pbs * pblk                      # B step width in centroids

    nc = bacc.Bacc(None, target_bir_lowering=False)
    A = nc.dram_tensor("a", [tt, PART, kt, PART], DT, kind="ExternalInput")
    B2 = nc.dram_tensor("b", [steps, kt, PART, bw], DT, kind="ExternalInput")
    C2 = nc.dram_tensor("c2", [PART, pb * pblk], F32, kind="ExternalInput")
    E2 = nc.dram_tensor("e2", [PART, tt], F32, kind="ExternalInput")
    OUT = nc.dram_tensor("out", [PART, tt], F32, kind="ExternalOutput")

    with ExitStack() as ctx:
        tc = ctx.enter_context(tile.TileContext(nc))
        const = ctx.enter_context(tc.tile_pool(name="const", bufs=1))
        c2_sb = const.tile([PART, pb * pblk], F32)
        e2_sb = const.tile([PART, tt], F32)
        out_sb = const.tile([PART, tt], F32)

        a_pool = ctx.enter_context(tc.tile_pool(name="apool", bufs=tt))
        b_pool = ctx.enter_context(tc.tile_pool(name="bpool", bufs=2))
        psum_pool = ctx.enter_context(tc.tile_pool(name="psum", bufs=4, space="PSUM"))
        mb_pool = ctx.enter_context(tc.tile_pool(name="mb", bufs=tt))
        m1_pool = ctx.enter_context(tc.tile_pool(name="m1", bufs=2))
        scr_pool = ctx.enter_context(tc.tile_pool(name="scr", bufs=3))

        a_tiles = [None] * tt
        minblks = [None] * tt

        # First token tiles ahead of the B stream so the PE can start ASAP.
        for t in range(min(2, tt)):
            a_tiles[t] = a_pool.tile([PART, kt, PART], DT, name="a_t", tag="a_t")
            nc.sync.dma_start(a_tiles[t][:], A[t])

        for q in range(steps):
            bq = b_pool.tile([PART, kt, bw], DT)
            # One batched strided DMA for the whole B step (fewer descriptor
            # round-trips than 12 row DMAs in the serialized startup window).
            for k in range(kt):
                nc.sync.dma_start(bq[:, k, :], B2[q, k])
            nc.sync.dma_start(c2_sb[:, q * bw : (q + 1) * bw],
                              C2[:, q * bw : (q + 1) * bw])
            if q == min(1, steps - 1):
                nc.sync.dma_start(e2_sb[:], E2[:])
            for t in range(tt):
                if q == 0:
                    if a_tiles[t] is None:
                        a_tiles[t] = a_pool.tile([PART, kt, PART], DT, name="a_t", tag="a_t")
                        nc.sync.dma_start(a_tiles[t][:], A[t])
                    minblks[t] = mb_pool.tile([PART, steps], F32, name="mb", tag="mb")
                a_t = a_tiles[t]
                # One 2-bank PSUM tile per (t, step): both centroid blocks of
                # the step land side by side, drained by ONE add + ONE min.
                ps = psum_pool.tile([PART, bw], F32)
                for j in range(pbs):
                    for kk in range(0, kt, kstep):
                        if fp8:
                            lhs = a_t[:, kk : kk + 2, :]
                            rhs = bq[:, kk : kk + 2, ts(j, pblk)]
                        else:
                            lhs = a_t[:, kk, :]
                            rhs = bq[:, kk, ts(j, pblk)]
                        nc.tensor.matmul(
                            ps[:, ts(j, pblk)], lhs, rhs,
                            start=(kk == 0),
                            stop=(kk + kstep >= kt),
                            perf_mode=perf_mode,
                        )
                # acc + c2_step -> SBUF scratch (frees the PSUM banks right
                # after the add instead of after the reduce, killing the
                # periodic PE stall on PSUM recycle); stepmin = min_free
                sc = scr_pool.tile([PART, bw], F32, name="sc", tag="sc")
                nc.vector.tensor_add(sc[:], ps[:], c2_sb[:, q * bw : (q + 1) * bw])
                nc.vector.tensor_reduce(
                    minblks[t][:, q : q + 1],
                    sc[:],
                    axis=mybir.AxisListType.X,
                    op=mybir.AluOpType.min,
                )
                if q == steps - 1:
                    m1 = m1_pool.tile([PART, 1], F32)
                    nc.vector.tensor_reduce(
                        m1[:], minblks[t][:], axis=mybir.AxisListType.X,
                        op=mybir.AluOpType.min,
                    )
                    nc.scalar.activation(
                        out_sb[:, t : t + 1], m1[:],
                        mybir.ActivationFunctionType.Sqrt,
                        bias=e2_sb[:, t : t + 1], scale=1.0,
                    )
        nc.sync.dma_start(OUT[:], out_sb[:])
    nc.compile()
    return nc


def prep_inputs(embeds, centroids, dtype=DEFAULT_DTYPE):
    """Host-side prep: scale/cast/layout so the device only does matmul+min+sqrt."""
    npdt = _NP_DT[dtype]
    steps = PB // PB_STEP
    bw = PB_STEP * PBLK

    e = np.ascontiguousarray(embeds, dtype=np.float32)
    c = np.ascontiguousarray(centroids, dtype=np.float32)

    # A[core][t, dp, k, m] = -2 * e[core, t*128+m, k*128+dp]
    ep = np.zeros((B, NPAD, D), dtype=npdt)
    ep[:, :N] = (e * -2.0).astype(npdt)
    a_all = np.ascontiguousarray(
        ep.reshape(B, TT, PART, KT, PART).transpose(0, 1, 4, 3, 2)
    )

    # B2[q, k, dp, j*512+i] = c[(q*PB_STEP+j)*512 + i, k*128 + dp]
    b2 = np.ascontiguousarray(
        c.reshape(steps, PB_STEP, PBLK, KT, PART)
        .transpose(0, 3, 4, 1, 2)
        .reshape(steps, KT, PART, bw)
        .astype(npdt)
    )

    # c2 broadcast to all partitions (fp32)
    c2 = np.einsum("pd,pd->p", c, c, dtype=np.float32)
    c2b = np.ascontiguousarray(np.broadcast_to(c2, (PART, P)))

    # e2[core][p, t] = ||e[core, t*128+p]||^2, padded tokens -> 0
    e2 = np.zeros((B, NPAD), dtype=np.float32)
    e2[:, :N] = np.einsum("bnd,bnd->bn", e, e, dtype=np.float32)
    e2t = np.ascontiguousarray(e2.reshape(B, TT, PART).transpose(0, 2, 1))

    return [
        {"a": a_all[b], "b": b2, "c2": c2b, "e2": e2t[b]} for b in range(B)
    ]


_NC_CACHE = {}


def _get_nc(dtype):
    if dtype not in _NC_CACHE:
        _NC_CACHE[dtype] = build_nc(dtype=dtype)
    return _NC_CACHE[dtype]


def run(inputs, trace=False, dtype=DEFAULT_DTYPE, **kwargs):
    """Returns ((loss, score), BassKernelResults)."""
    nc = _get_nc(dtype)
    in_maps = prep_inputs(inputs["embeds"], inputs["centroids"], dtype=dtype)
    res = run_bass_kernel_spmd(nc, in_maps, core_ids=list(range(NCORES)),
                               trace=trace, **kwargs)
    outs = np.stack([np.asarray(res.results[i]["out"]) for i in range(NCORES)])
    # out[p, t] -> token index t*128 + p
    toks = outs.transpose(0, 2, 1).reshape(B, NPAD)[:, :N]
    score = toks.reshape(B, 1, H, H).astype(np.float32)
    loss = np.float32(0.0)
    return (loss, score), res


def kernel(**inputs):
    (loss, score), _ = run(inputs)
    return loss, score
